# revision 1
# baseline (speedup 1.0000x reference)
"""AuthPct metric kernel for 8 Trainium2 NeuronCores.

Sharding: real_stats rows are sharded across the 8 cores (1536 each).
For column features f_j each core computes PSUM tiles of

    Y[j, i] = 2*f_j.r_i - |r_i|^2 - |f_j|^2  =  -dist^2(f_j, r_i)

via bf16 PE matmuls: two K=128 feature chunks plus one augmented K=128
matmul (rows 0..3 of its operands carry the exact hi/lo bf16 splits of
-|r_i|^2 and -|f_j|^2; remaining rows are zero).  For the gen side the
aug lhsT has only the |r_i|^2 rows, so gen tiles hold X = 2G - |r_i|^2.

gen (96 j-tiles/core, all gen columns vs core rows): ScalarE copies
PSUM->SBUF wide [128,1536]; DVE `max` top-8 (d1 values) + `max_index`
(argmin payload for d2).  The 60 real j-tiles are spread evenly over
the 96 loop slots so the DVE (bottleneck) sees uniform demand.

real: the distance matrix is symmetric, so each unordered shard pair is
computed once.  With host-rotated real columns every core runs the SAME
program on j-tiles covering shards c..c+4 (60 j-tiles): DVE `max` top-8
gives the j-side min (the diagonal lands in the self block where
Y_diag ~ 0 while true neighbors are ~ -300, so host uses top-2 there);
Pool `partition_all_reduce(max)` on blocks c+1..c+4 gives the i-side
min over each tile's 128 j's, and the j-side DVE scans run only on
blocks c..c+3 (free-side coverage s-3..s plus PAR coverage s+1..s+4
spans all 8 shards).  This cuts real-side DVE scans from 96 to 48
j-tiles; the partition reduces ride on the otherwise-idle Pool engine.
Measured ~429 us HW exec; the DVE scans run back-to-back (~98% busy,
<1 us total idle), so the span is the scan floor plus ramp and the
fixed Tile tail barrier.

Host combines the per-core partials (min over all candidates), gathers
d2 = realNN[argmin], applies sigmoid and the mean.  All reductions are
exact fp32; only the Gram matmuls are bf16.
"""

import numpy as np

N = 12288
D = 256
NCORES = 8
SHARD = N // NCORES          # 1536 rows per core
JTILE = 128                  # j columns per tile (PSUM partitions)
NJT = N // JTILE             # 96 gen j-tiles
RJT = 60                     # real j-tiles: shards c..c+4 (rotated)
FJT = 48                     # j-tiles with a DVE free-side scan (m=0..3)
PAR_LO, PAR_HI = 12, 60      # real j-tiles with partition-reduce harvest
NT = 512                     # i elements per matmul (PSUM bank)
NIT = SHARD // NT            # 3 i-tiles

_cached_nc = None


def _build_nc():
    import concourse.bass_isa as bass_isa
    import concourse.mybir as mybir
    from concourse import bacc
    from concourse.tile import TileContext

    f32 = mybir.dt.float32
    bf16 = mybir.dt.bfloat16
    u32 = mybir.dt.uint32

    nc = bacc.Bacc("TRN2", target_bir_lowering=False, debug=False,
                   num_devices=NCORES)

    colr = nc.dram_tensor("colr", [D, RJT * JTILE], bf16,
                          kind="ExternalInput")
    colg = nc.dram_tensor("colg", [D, N], bf16, kind="ExternalInput")
    auglr = nc.dram_tensor("auglr", [JTILE, RJT * JTILE], bf16,
                           kind="ExternalInput")
    rhs = nc.dram_tensor("rhs", [D, SHARD], bf16, kind="ExternalInput")
    aug = nc.dram_tensor("aug", [128, SHARD], bf16, kind="ExternalInput")
    ones = nc.dram_tensor("ones", [JTILE, JTILE], bf16, kind="ExternalInput")

    o_realv = nc.dram_tensor("o_realv", [128, FJT * 8], f32,
                             kind="ExternalOutput")
    o_par = nc.dram_tensor("o_par", [PAR_HI - PAR_LO, SHARD], f32,
                           kind="ExternalOutput")
    o_genv = nc.dram_tensor("o_genv", [128, NJT * 8], f32,
                            kind="ExternalOutput")
    o_geni = nc.dram_tensor("o_geni", [128, NJT * 8], u32,
                            kind="ExternalOutput")

    with TileContext(nc) as tc:
        with (
            tc.tile_pool(name="const", bufs=1) as constp,
            tc.tile_pool(name="lhs", bufs=6) as lhsp,
            tc.tile_pool(name="wide", bufs=6) as widep,
            tc.tile_pool(name="parp", bufs=3) as parp,
            tc.tile_pool(name="outb", bufs=1) as outp,
            tc.tile_pool(name="ps", bufs=8, space="PSUM") as psp,
        ):
            # Resident rhs: both K-chunks of 2*realT shard, in per-i-tile
            # slices so the first matmul group starts early.
            rhs_sb = constp.tile([128, 2 * SHARD], bf16)
            nc.sync.dma_start(out=rhs_sb[:, 0:NT], in_=rhs[0:128, 0:NT])
            nc.sync.dma_start(out=rhs_sb[:, SHARD:SHARD + NT],
                              in_=rhs[128:256, 0:NT])
            # aug rhs rows: 0,1 = -hi/lo(|r_i|^2); 2,3 = 1.0; rest zero,
            # fully materialized host-side (zero-padded to K=128: a K<128
            # matmul stalls the PE pipeline; a device-side memset would
            # serialize the first aug matmul behind Pool)
            aug_sb = constp.tile([128, SHARD], bf16)
            nc.sync.dma_start(out=aug_sb[:, 0:NT], in_=aug[:, 0:NT])
            ones_sb = constp.tile([JTILE, JTILE], bf16)
            nc.sync.dma_start(out=ones_sb[:, :], in_=ones[:, :])

            realv = outp.tile([128, FJT * 8], f32)
            genv = outp.tile([128, NJT * 8], f32)
            geni = outp.tile([128, NJT * 8], u32)

            for jt in range(NJT):
                jo = jt * JTILE
                # spread the 60 real tiles evenly over the 96 slots so the
                # DVE load per slot is uniform
                do_real = (jt * RJT) // NJT != ((jt + 1) * RJT) // NJT
                rjt = (jt * RJT) // NJT
                jor = rjt * JTILE
                lhs_g = lhsp.tile([128, 2 * JTILE], bf16, tag="lhs_g")
                nc.sync.dma_start(
                    out=lhs_g[:, :].rearrange("p (c j) -> p c j", c=2),
                    in_=colg[:, jo:jo + JTILE].rearrange(
                        "(c p) j -> p c j", c=2),
                )
                if jt == 0:
                    # remaining const slices, behind jt0's critical loads
                    for it0 in range(1, NIT):
                        io0 = it0 * NT
                        nc.sync.dma_start(out=rhs_sb[:, io0:io0 + NT],
                                          in_=rhs[0:128, io0:io0 + NT])
                        nc.sync.dma_start(
                            out=rhs_sb[:, SHARD + io0:SHARD + io0 + NT],
                            in_=rhs[128:256, io0:io0 + NT])
                        nc.sync.dma_start(out=aug_sb[:, io0:io0 + NT],
                                          in_=aug[:, io0:io0 + NT])
                if do_real:
                    lhs_r = lhsp.tile([128, 2 * JTILE], bf16, tag="lhs_r")
                    nc.sync.dma_start(
                        out=lhs_r[:, :].rearrange("p (c j) -> p c j", c=2),
                        in_=colr[:, jor:jor + JTILE].rearrange(
                            "(c p) j -> p c j", c=2),
                    )
                    auglr_t = lhsp.tile([128, JTILE], bf16, tag="auglr_t")
                    nc.sync.dma_start(out=auglr_t[:, :],
                                      in_=auglr[:, jor:jor + JTILE])

                wide_g = widep.tile([128, SHARD], f32, tag="wide_g")
                if do_real:
                    wide_r = widep.tile([128, SHARD], f32, tag="wide_r")

                for it in range(NIT):
                    io = it * NT
                    jobs = [(lhs_g, ones_sb, wide_g)]
                    if do_real:
                        jobs.append((lhs_r, auglr_t, wide_r))
                    for lhs_t, aug_lhs, wide in jobs:
                        ps = psp.tile([128, NT], f32)
                        nc.tensor.matmul(
                            out=ps[:, :],
                            lhsT=lhs_t[:, 0:JTILE],
                            rhs=rhs_sb[:, io:io + NT],
                            start=True, stop=False,
                        )
                        nc.tensor.matmul(
                            out=ps[:, :],
                            lhsT=lhs_t[:, JTILE:2 * JTILE],
                            rhs=rhs_sb[:, SHARD + io:SHARD + io + NT],
                            start=False, stop=False,
                        )
                        nc.tensor.matmul(
                            out=ps[:, :],
                            lhsT=aug_lhs[:, :],
                            rhs=aug_sb[:, io:io + NT],
                            start=False, stop=True,
                        )
                        nc.scalar.activation(
                            out=wide[:, io:io + NT],
                            in_=ps[:, :],
                            func=mybir.ActivationFunctionType.Copy,
                        )

                nc.vector.max(out=genv[:, jt * 8:(jt + 1) * 8],
                              in_=wide_g[:, :])
                nc.vector.max_index(out=geni[:, jt * 8:(jt + 1) * 8],
                                    in_max=genv[:, jt * 8:(jt + 1) * 8],
                                    in_values=wide_g[:, :])
                if do_real:
                    if rjt < 12:
                        # self block: top-8 (host drops the diagonal top-1)
                        nc.vector.max(out=realv[:, rjt * 8:(rjt + 1) * 8],
                                      in_=wide_r[:, :])
                    elif rjt < FJT:
                        # only the max is needed: plain reduce is cheaper
                        nc.vector.tensor_reduce(
                            out=realv[:, rjt * 8:rjt * 8 + 1],
                            in_=wide_r[:, :],
                            axis=mybir.AxisListType.X,
                            op=mybir.AluOpType.max)
                    if PAR_LO <= rjt < PAR_HI:
                        par_t = parp.tile([128, SHARD], f32, tag="par_t")
                        nc.gpsimd.partition_all_reduce(
                            par_t[:, :], wide_r[:, :], channels=128,
                            reduce_op=bass_isa.ReduceOp.max)
                        nc.sync.dma_start(
                            out=o_par[rjt - PAR_LO:rjt - PAR_LO + 1, :],
                            in_=par_t[0:1, :])

            nc.sync.dma_start(out=o_realv[:, :], in_=realv[:, :])
            nc.sync.dma_start(out=o_genv[:, :], in_=genv[:, :])
            nc.sync.dma_start(out=o_geni[:, :], in_=geni[:, :])

    nc.compile()
    return nc


def _hilo(x, bf):
    hi = x.astype(bf)
    lo = (x - hi.astype(np.float32)).astype(bf)
    return hi, lo


def kernel(real_stats, gen_stats, _trace=False):
    import ml_dtypes
    from concourse.bass_utils import run_bass_kernel_spmd

    bf = ml_dtypes.bfloat16
    global _cached_nc
    real = np.ascontiguousarray(np.asarray(real_stats, dtype=np.float32))
    gen = np.ascontiguousarray(np.asarray(gen_stats, dtype=np.float32))

    realT = np.ascontiguousarray(real.T)                  # [D, N]
    genT = np.ascontiguousarray(gen.T)
    colg_bf = genT.astype(bf)
    rhs_bf = (2.0 * realT).astype(bf)                     # [D, N]
    b2 = np.sum(real.astype(np.float64) ** 2, axis=1).astype(np.float32)
    a2g = np.sum(gen.astype(np.float64) ** 2, axis=1).astype(np.float32)
    ones = np.zeros((JTILE, JTILE), dtype=bf)
    ones[0:2, :] = 1

    RW = RJT * JTILE                                      # 7680 rotated cols
    in_maps = []
    for c in range(NCORES):
        sl = slice(c * SHARD, (c + 1) * SHARD)
        negb2_hi, negb2_lo = _hilo(-b2[sl], bf)
        aug4 = np.zeros((128, SHARD), dtype=bf)
        aug4[0] = negb2_hi
        aug4[1] = negb2_lo
        aug4[2:4] = 1
        colr_rot = np.roll(realT, -c * SHARD, axis=1)[:, :RW]
        a2rot = np.roll(b2, -c * SHARD)[:RW]
        nega2_hi, nega2_lo = _hilo(-a2rot, bf)
        auglr = np.zeros((JTILE, RW), dtype=bf)
        auglr[0:2] = 1
        auglr[2] = nega2_hi
        auglr[3] = nega2_lo
        in_maps.append({
            "colr": colr_rot.astype(bf),
            "colg": colg_bf,
            "auglr": auglr,
            "rhs": np.ascontiguousarray(rhs_bf[:, sl]),
            "aug": aug4,
            "ones": ones,
        })

    if _cached_nc is None:
        _cached_nc = _build_nc()
    res = run_bass_kernel_spmd(_cached_nc, in_maps,
                               core_ids=list(range(NCORES)),
                               trace=_trace)

    # ---- host combine ----
    def grid(name, c, width):
        # [128, width*8] -> [128, width, 8]
        return res.results[c][name].reshape(128, width, 8)

    # real: Y = -dist^2 candidates, min-combined over all sources
    cand = np.full(N, np.inf, dtype=np.float64)
    p_idx = np.arange(128)
    for c in range(NCORES):
        rv = grid("o_realv", c, FJT)                      # [128, FJT, 8] of Y
        top1 = rv[:, :, 0]
        top2 = rv[:, :, 1]
        # self block (k < 12) contains the diagonal: Y_diag ~ 0, true
        # neighbors ~ -300 -> take top2 there when top1 is diag-like
        use2 = np.zeros((128, FJT), dtype=bool)
        use2[:, :12] = top1[:, :12] > -10.0
        y = np.where(use2, top2, top1)                    # [128, FJT]
        jglob = (c * SHARD + np.arange(FJT)[None, :] * JTILE
                 + p_idx[:, None]) % N
        np.minimum.at(cand, jglob.ravel(), (-y).ravel())
        par = res.results[c]["o_par"]                     # [48, SHARD] of Y
        par_min = -par.max(axis=0)                        # min dist^2 per i
        sl = slice(c * SHARD, (c + 1) * SHARD)
        cand[sl] = np.minimum(cand[sl], par_min)
    realNN = np.sqrt(np.maximum(cand, 0.0))               # [N]

    # gen: X = 2G - |r_i|^2;  d1^2 = a2g - max X
    j = np.arange(N)
    genv = np.stack([grid("o_genv", c, NJT)[:, :, 0] for c in range(NCORES)])
    geni = np.stack([res.results[c]["o_geni"].reshape(128, NJT, 8)[:, :, 0]
                     for c in range(NCORES)])
    # [8, 128, NJT] -> [8, N] with j = jt*128 + p
    gv = genv.transpose(0, 2, 1).reshape(NCORES, N)
    gi = geni.transpose(0, 2, 1).reshape(NCORES, N)
    cstar = gv.argmax(axis=0)
    d1 = np.sqrt(np.maximum(a2g - gv[cstar, j], 0.0))
    istar = cstar * SHARD + gi[cstar, j]
    d2 = realNN[istar]

    z = (d2 - d1) / 0.1
    authen = np.where(z >= 0, 1.0 / (1.0 + np.exp(-np.abs(z))),
                      np.exp(-np.abs(z)) / (1.0 + np.exp(-np.abs(z))))
    out = np.asarray(-100.0 * np.mean(authen), dtype=np.float32)
    if _trace:
        return out, res
    return out



# revision 2
# speedup vs baseline: 1.2453x; 1.2453x over previous
"""AuthPct metric kernel for 8 Trainium2 NeuronCores.

Sharding: real_stats rows are sharded across the 8 cores (1536 each).
For column features f_j each core computes PSUM tiles of

    Y[j, i] = 2*f_j.r_i - |r_i|^2 - |f_j|^2  =  -dist^2(f_j, r_i)

via bf16 PE matmuls: two K=128 feature chunks plus one augmented K=128
matmul (rows 0..3 of its operands carry the exact hi/lo bf16 splits of
-|r_i|^2 and -|f_j|^2; remaining rows are zero).  For the gen side the
aug lhsT has only the |r_i|^2 rows, so gen tiles hold X = 2G - |r_i|^2.

gen (96 j-tiles/core): the three i-banks accumulate into one 3-bank
PSUM tile [128,1536]; the DVE reduces it PSUM-direct with a segmented
tensor_reduce into 12 per-128-block maxima of X (no ScalarE copy, no
max_index).  The host finds each gen column's best block, refines every
block within a small margin exactly (tiny numpy gemms) and recovers the
exact argmin index + d1.  This removes the FIND_INDEX8 pass and the gen
MAX8 pass that made the DVE the 96%-busy bottleneck of v1 (429 us).

real: the distance matrix is symmetric, so each unordered shard pair is
computed once.  With host-rotated real columns every core runs the SAME
program on j-tiles covering shards c..c+4 (60 j-tiles): DVE `max` top-8
gives the j-side min (the diagonal lands in the self block where
Y_diag ~ 0 while true neighbors are ~ -300, so host uses top-2 there);
Pool `partition_all_reduce(max)` on blocks c+1..c+4 gives the i-side
min over each tile's 128 j's, and the j-side DVE scans run only on
blocks c..c+3 (free-side coverage s-3..s plus PAR coverage s+1..s+4
spans all 8 shards).

Host combines the per-core partials (min over all candidates), gathers
d2 = realNN[argmin], applies sigmoid and the mean.  All reductions are
exact fp32; only the Gram matmuls are bf16.
"""

import numpy as np

N = 12288
D = 256
NCORES = 8
SHARD = N // NCORES          # 1536 rows per core
JTILE = 128                  # j columns per tile (PSUM partitions)
NJT = N // JTILE             # 96 gen j-tiles
RJT = 60                     # real j-tiles: shards c..c+4 (rotated)
FJT = 48                     # j-tiles with a DVE free-side scan (m=0..3)
PAR_LO, PAR_HI = 12, 60      # real j-tiles with partition-reduce harvest
NT = 512                     # i elements per matmul (PSUM bank)
NIT = SHARD // NT            # 3 i-tiles
NBLK = SHARD // JTILE        # 12 i-blocks of 128 per core (gen block minima)
MARGIN = 1.5                 # d^2 margin for host argmin refinement (bf16)

_cached_nc = None


def _build_nc():
    import concourse.bass_isa as bass_isa
    import concourse.mybir as mybir
    from concourse import bacc
    from concourse.tile import TileContext

    f32 = mybir.dt.float32
    bf16 = mybir.dt.bfloat16

    nc = bacc.Bacc("TRN2", target_bir_lowering=False, debug=False,
                   num_devices=NCORES)

    colr = nc.dram_tensor("colr", [D, RJT * JTILE], bf16,
                          kind="ExternalInput")
    colg = nc.dram_tensor("colg", [D, N], bf16, kind="ExternalInput")
    auglr = nc.dram_tensor("auglr", [JTILE, RJT * JTILE], bf16,
                           kind="ExternalInput")
    rhs = nc.dram_tensor("rhs", [D, SHARD], bf16, kind="ExternalInput")
    aug = nc.dram_tensor("aug", [128, SHARD], bf16, kind="ExternalInput")
    ones = nc.dram_tensor("ones", [JTILE, JTILE], bf16, kind="ExternalInput")

    o_realv = nc.dram_tensor("o_realv", [128, FJT * 8], f32,
                             kind="ExternalOutput")
    o_par = nc.dram_tensor("o_par", [PAR_HI - PAR_LO, SHARD], f32,
                           kind="ExternalOutput")
    o_genv = nc.dram_tensor("o_genv", [128, NJT * NBLK], f32,
                            kind="ExternalOutput")

    with TileContext(nc) as tc:
        with (
            tc.tile_pool(name="const", bufs=1) as constp,
            tc.tile_pool(name="lhs", bufs=6) as lhsp,
            tc.tile_pool(name="wide", bufs=4) as widep,
            tc.tile_pool(name="parp", bufs=3) as parp,
            tc.tile_pool(name="outb", bufs=1) as outp,
            tc.tile_pool(name="psg", bufs=2, space="PSUM") as psgp,
            tc.tile_pool(name="psr", bufs=2, space="PSUM") as psrp,
        ):
            # Resident rhs: both K-chunks of 2*realT shard, in per-i-tile
            # slices so the first matmul group starts early.
            rhs_sb = constp.tile([128, 2 * SHARD], bf16)
            nc.sync.dma_start(out=rhs_sb[:, 0:NT], in_=rhs[0:128, 0:NT])
            nc.sync.dma_start(out=rhs_sb[:, SHARD:SHARD + NT],
                              in_=rhs[128:256, 0:NT])
            # aug rhs rows: 0,1 = -hi/lo(|r_i|^2); 2,3 = 1.0; rest zero,
            # fully materialized host-side (zero-padded to K=128: a K<128
            # matmul stalls the PE pipeline; a device-side memset would
            # serialize the first aug matmul behind Pool)
            aug_sb = constp.tile([128, SHARD], bf16)
            nc.sync.dma_start(out=aug_sb[:, 0:NT], in_=aug[:, 0:NT])
            ones_sb = constp.tile([JTILE, JTILE], bf16)
            nc.sync.dma_start(out=ones_sb[:, :], in_=ones[:, :])

            realv = outp.tile([128, FJT * 8], f32)
            genv = outp.tile([128, NJT * NBLK], f32)

            for jt in range(NJT):
                jo = jt * JTILE
                # spread the 60 real tiles evenly over the 96 slots so the
                # DVE load per slot is uniform
                do_real = (jt * RJT) // NJT != ((jt + 1) * RJT) // NJT
                rjt = (jt * RJT) // NJT
                jor = rjt * JTILE
                lhs_g = lhsp.tile([128, 2 * JTILE], bf16, tag="lhs_g")
                nc.sync.dma_start(
                    out=lhs_g[:, :].rearrange("p (c j) -> p c j", c=2),
                    in_=colg[:, jo:jo + JTILE].rearrange(
                        "(c p) j -> p c j", c=2),
                )
                if jt == 0:
                    # remaining const slices, behind jt0's critical loads
                    for it0 in range(1, NIT):
                        io0 = it0 * NT
                        nc.sync.dma_start(out=rhs_sb[:, io0:io0 + NT],
                                          in_=rhs[0:128, io0:io0 + NT])
                        nc.sync.dma_start(
                            out=rhs_sb[:, SHARD + io0:SHARD + io0 + NT],
                            in_=rhs[128:256, io0:io0 + NT])
                        nc.sync.dma_start(out=aug_sb[:, io0:io0 + NT],
                                          in_=aug[:, io0:io0 + NT])
                if do_real:
                    lhs_r = lhsp.tile([128, 2 * JTILE], bf16, tag="lhs_r")
                    nc.sync.dma_start(
                        out=lhs_r[:, :].rearrange("p (c j) -> p c j", c=2),
                        in_=colr[:, jor:jor + JTILE].rearrange(
                            "(c p) j -> p c j", c=2),
                    )
                    auglr_t = lhsp.tile([128, JTILE], bf16, tag="auglr_t")
                    nc.sync.dma_start(out=auglr_t[:, :],
                                      in_=auglr[:, jor:jor + JTILE])
                    wide_r = widep.tile([128, SHARD], f32, tag="wide_r")

                # gen: one 3-bank PSUM tile, reduced PSUM-direct by the DVE
                ps_g = psgp.tile([128, SHARD], f32, tag="ps_g")
                for it in range(NIT):
                    io = it * NT
                    nc.tensor.matmul(
                        out=ps_g[:, io:io + NT],
                        lhsT=lhs_g[:, 0:JTILE],
                        rhs=rhs_sb[:, io:io + NT],
                        start=True, stop=False,
                    )
                    nc.tensor.matmul(
                        out=ps_g[:, io:io + NT],
                        lhsT=lhs_g[:, JTILE:2 * JTILE],
                        rhs=rhs_sb[:, SHARD + io:SHARD + io + NT],
                        start=False, stop=False,
                    )
                    nc.tensor.matmul(
                        out=ps_g[:, io:io + NT],
                        lhsT=ones_sb[:, :],
                        rhs=aug_sb[:, io:io + NT],
                        start=False, stop=True,
                    )
                    if do_real:
                        io2 = io
                        ps_r = psrp.tile([128, NT], f32, tag="ps_r")
                        nc.tensor.matmul(
                            out=ps_r[:, :],
                            lhsT=lhs_r[:, 0:JTILE],
                            rhs=rhs_sb[:, io2:io2 + NT],
                            start=True, stop=False,
                        )
                        nc.tensor.matmul(
                            out=ps_r[:, :],
                            lhsT=lhs_r[:, JTILE:2 * JTILE],
                            rhs=rhs_sb[:, SHARD + io2:SHARD + io2 + NT],
                            start=False, stop=False,
                        )
                        nc.tensor.matmul(
                            out=ps_r[:, :],
                            lhsT=auglr_t[:, :],
                            rhs=aug_sb[:, io2:io2 + NT],
                            start=False, stop=True,
                        )
                        nc.scalar.activation(
                            out=wide_r[:, io2:io2 + NT],
                            in_=ps_r[:, :],
                            func=mybir.ActivationFunctionType.Copy,
                        )

                # per-128-block maxima of X, straight out of PSUM
                nc.vector.tensor_reduce(
                    out=genv[:, jt * NBLK:(jt + 1) * NBLK],
                    in_=ps_g[:, :].rearrange("p (b x) -> p b x", b=NBLK),
                    axis=mybir.AxisListType.X,
                    op=mybir.AluOpType.max)

                if do_real:
                    if rjt < 12:
                        # self block: top-8 (host drops the diagonal top-1)
                        nc.vector.max(out=realv[:, rjt * 8:(rjt + 1) * 8],
                                      in_=wide_r[:, :])
                    elif rjt < FJT:
                        # only the max is needed: plain reduce is cheaper
                        nc.vector.tensor_reduce(
                            out=realv[:, rjt * 8:rjt * 8 + 1],
                            in_=wide_r[:, :],
                            axis=mybir.AxisListType.X,
                            op=mybir.AluOpType.max)
                    if PAR_LO <= rjt < PAR_HI:
                        par_t = parp.tile([128, SHARD], f32, tag="par_t")
                        nc.gpsimd.partition_all_reduce(
                            par_t[:, :], wide_r[:, :], channels=128,
                            reduce_op=bass_isa.ReduceOp.max)
                        nc.sync.dma_start(
                            out=o_par[rjt - PAR_LO:rjt - PAR_LO + 1, :],
                            in_=par_t[0:1, :])

            nc.sync.dma_start(out=o_realv[:, :], in_=realv[:, :])
            nc.sync.dma_start(out=o_genv[:, :], in_=genv[:, :])

    nc.compile()
    return nc


def _hilo(x, bf):
    hi = x.astype(bf)
    lo = (x - hi.astype(np.float32)).astype(bf)
    return hi, lo


def kernel(real_stats, gen_stats, _trace=False):
    import ml_dtypes
    from concourse.bass_utils import run_bass_kernel_spmd

    bf = ml_dtypes.bfloat16
    global _cached_nc
    real = np.ascontiguousarray(np.asarray(real_stats, dtype=np.float32))
    gen = np.ascontiguousarray(np.asarray(gen_stats, dtype=np.float32))

    realT = np.ascontiguousarray(real.T)                  # [D, N]
    genT = np.ascontiguousarray(gen.T)
    colg_bf = genT.astype(bf)
    rhs_bf = (2.0 * realT).astype(bf)                     # [D, N]
    b2 = np.sum(real.astype(np.float64) ** 2, axis=1).astype(np.float32)
    a2g = np.sum(gen.astype(np.float64) ** 2, axis=1).astype(np.float32)
    ones = np.zeros((JTILE, JTILE), dtype=bf)
    ones[0:2, :] = 1

    RW = RJT * JTILE                                      # 7680 rotated cols
    in_maps = []
    for c in range(NCORES):
        sl = slice(c * SHARD, (c + 1) * SHARD)
        negb2_hi, negb2_lo = _hilo(-b2[sl], bf)
        aug4 = np.zeros((128, SHARD), dtype=bf)
        aug4[0] = negb2_hi
        aug4[1] = negb2_lo
        aug4[2:4] = 1
        colr_rot = np.roll(realT, -c * SHARD, axis=1)[:, :RW]
        a2rot = np.roll(b2, -c * SHARD)[:RW]
        nega2_hi, nega2_lo = _hilo(-a2rot, bf)
        auglr = np.zeros((JTILE, RW), dtype=bf)
        auglr[0:2] = 1
        auglr[2] = nega2_hi
        auglr[3] = nega2_lo
        in_maps.append({
            "colr": colr_rot.astype(bf),
            "colg": colg_bf,
            "auglr": auglr,
            "rhs": np.ascontiguousarray(rhs_bf[:, sl]),
            "aug": aug4,
            "ones": ones,
        })

    if _cached_nc is None:
        _cached_nc = _build_nc()
    res = run_bass_kernel_spmd(_cached_nc, in_maps,
                               core_ids=list(range(NCORES)),
                               trace=_trace)

    # ---- host combine ----
    # real: Y = -dist^2 candidates, min-combined over all sources
    cand = np.full(N, np.inf, dtype=np.float64)
    p_idx = np.arange(128)
    for c in range(NCORES):
        rv = res.results[c]["o_realv"].reshape(128, FJT, 8)
        top1 = rv[:, :, 0]
        top2 = rv[:, :, 1]
        # self block (k < 12) contains the diagonal: Y_diag ~ 0, true
        # neighbors ~ -300 -> take top2 there when top1 is diag-like
        use2 = np.zeros((128, FJT), dtype=bool)
        use2[:, :12] = top1[:, :12] > -10.0
        y = np.where(use2, top2, top1)                    # [128, FJT]
        jglob = (c * SHARD + np.arange(FJT)[None, :] * JTILE
                 + p_idx[:, None]) % N
        np.minimum.at(cand, jglob.ravel(), (-y).ravel())
        par = res.results[c]["o_par"]                     # [48, SHARD] of Y
        par_min = -par.max(axis=0)                        # min dist^2 per i
        sl = slice(c * SHARD, (c + 1) * SHARD)
        cand[sl] = np.minimum(cand[sl], par_min)
    realNN = np.sqrt(np.maximum(cand, 0.0))               # [N]

    # gen: X = 2G - |r_i|^2 block maxima -> host argmin refinement.
    # Xb[g, j]: per-gen-column max of X over global 128-block g.
    Xb = np.empty((NCORES * NBLK, N), dtype=np.float32)
    for c in range(NCORES):
        gv = res.results[c]["o_genv"].reshape(128, NJT, NBLK)
        # j = jt*128 + p ; global block = c*NBLK + b
        Xb[c * NBLK:(c + 1) * NBLK, :] = (
            gv.transpose(2, 1, 0).reshape(NBLK, N))
    best = Xb.max(axis=0)                                 # [N] max X
    # refine every block whose coarse max is within MARGIN of the best
    cand_mask = Xb >= (best - MARGIN)[None, :]            # [96, N]
    Xstar = np.full(N, -np.inf, dtype=np.float32)
    istar = np.zeros(N, dtype=np.int64)
    genf = gen.astype(np.float32)
    for g in range(NCORES * NBLK):
        js = np.nonzero(cand_mask[g])[0]
        if js.size == 0:
            continue
        rb = real[g * JTILE:(g + 1) * JTILE]              # [128, D]
        Xex = 2.0 * (genf[js] @ rb.T) - b2[g * JTILE:(g + 1) * JTILE][None, :]
        loc = np.argmax(Xex, axis=1)
        val = Xex[np.arange(js.size), loc]
        upd = val > Xstar[js]
        Xstar[js[upd]] = val[upd]
        istar[js[upd]] = g * JTILE + loc[upd]
    d1 = np.sqrt(np.maximum(a2g - Xstar, 0.0))
    d2 = realNN[istar]

    z = (d2 - d1) / 0.1
    authen = np.where(z >= 0, 1.0 / (1.0 + np.exp(-np.abs(z))),
                      np.exp(-np.abs(z)) / (1.0 + np.exp(-np.abs(z))))
    out = np.asarray(-100.0 * np.mean(authen), dtype=np.float32)
    if _trace:
        return out, res
    return out


# revision 9
# speedup vs baseline: 1.2561x; 1.0087x over previous
"""AuthPct metric kernel for 8 Trainium2 NeuronCores.

Sharding: real_stats rows are sharded across the 8 cores (1536 each).
For column features f_j each core computes PSUM tiles of

    Y[j, i] = 2*f_j.r_i - |r_i|^2 - |f_j|^2  =  -dist^2(f_j, r_i)

via bf16 PE matmuls: two K=128 feature chunks plus one augmented K=128
matmul (rows 0..3 of its operands carry the exact hi/lo bf16 splits of
-|r_i|^2 and -|f_j|^2; remaining rows are zero).  For the gen side the
aug lhsT has only the |r_i|^2 rows, so gen tiles hold X = 2G - |r_i|^2.

gen (96 j-tiles/core): the three i-banks accumulate into one 3-bank
PSUM tile [128,1536]; the DVE reduces it PSUM-direct with a segmented
tensor_reduce into 12 per-128-block maxima of X (no ScalarE copy, no
max_index).  The host finds each gen column's best block, refines every
block within a small margin exactly (tiny numpy gemms) and recovers the
exact argmin index + d1.  This removes the FIND_INDEX8 pass and the gen
MAX8 pass that made the DVE the 96%-busy bottleneck of v1 (429 us).

real: the distance matrix is symmetric, so each unordered shard pair is
computed once.  With host-rotated real columns every core runs the SAME
program on j-tiles covering shards c..c+4 (60 j-tiles): DVE `max` top-8
gives the j-side min (the diagonal lands in the self block where
Y_diag ~ 0 while true neighbors are ~ -300, so host uses top-2 there);
Pool `partition_all_reduce(max)` on blocks c+1..c+4 gives the i-side
min over each tile's 128 j's, and the j-side DVE scans run only on
blocks c..c+3 (free-side coverage s-3..s plus PAR coverage s+1..s+4
spans all 8 shards).

Host combines the per-core partials (min over all candidates), gathers
d2 = realNN[argmin], applies sigmoid and the mean.  All reductions are
exact fp32; only the Gram matmuls are bf16.
"""

import numpy as np

N = 12288
D = 256
NCORES = 8
SHARD = N // NCORES          # 1536 rows per core
JTILE = 128                  # j columns per tile (PSUM partitions)
NJT = N // JTILE             # 96 gen j-tiles
RJT = 60                     # real j-tiles: shards c..c+4 (rotated)
FJT = 48                     # j-tiles with a DVE free-side scan (m=0..3)
PAR_LO, PAR_HI = 12, 60      # real j-tiles with partition-reduce harvest
NT = 512                     # i elements per matmul (PSUM bank)
NIT = SHARD // NT            # 3 i-tiles
NBLK = SHARD // JTILE        # 12 i-blocks of 128 per core (gen block minima)
MARGIN = 6.0                 # d^2 margin for host argmin refinement (fp8)

_cached_nc = None


def _build_nc():
    import concourse.bass_isa as bass_isa
    import concourse.mybir as mybir
    from concourse import bacc
    from concourse.tile import TileContext

    f32 = mybir.dt.float32
    bf16 = mybir.dt.bfloat16
    fp8 = mybir.dt.float8e4

    nc = bacc.Bacc("TRN2", target_bir_lowering=False, debug=False,
                   num_devices=NCORES)

    colr = nc.dram_tensor("colr", [D, RJT * JTILE], bf16,
                          kind="ExternalInput")
    # gen columns in fp8 DoubleRow layout: [p, jt, t, j] = genT[t*128+p, .]
    colg8 = nc.dram_tensor("colg8", [128, NJT * 2 * JTILE], fp8,
                           kind="ExternalInput")
    rhs8 = nc.dram_tensor("rhs8", [128, 2 * SHARD], fp8,
                          kind="ExternalInput")
    auglr = nc.dram_tensor("auglr", [JTILE, RJT * JTILE], bf16,
                           kind="ExternalInput")
    rhs = nc.dram_tensor("rhs", [D, SHARD], bf16, kind="ExternalInput")
    aug = nc.dram_tensor("aug", [128, SHARD], bf16, kind="ExternalInput")
    ones = nc.dram_tensor("ones", [JTILE, JTILE], bf16, kind="ExternalInput")

    o_realv = nc.dram_tensor("o_realv", [128, FJT * 8], f32,
                             kind="ExternalOutput")
    o_par = nc.dram_tensor("o_par", [PAR_HI - PAR_LO, SHARD], f32,
                           kind="ExternalOutput")
    o_genv = nc.dram_tensor("o_genv", [128, NJT * NBLK], f32,
                            kind="ExternalOutput")

    with TileContext(nc) as tc:
        with (
            tc.tile_pool(name="const", bufs=1) as constp,
            tc.tile_pool(name="lhs", bufs=6) as lhsp,
            tc.tile_pool(name="wide", bufs=4) as widep,
            tc.tile_pool(name="parp", bufs=3) as parp,
            tc.tile_pool(name="outb", bufs=1) as outp,
            tc.tile_pool(name="psg", bufs=2, space="PSUM") as psgp,
            tc.tile_pool(name="psr", bufs=2, space="PSUM") as psrp,
        ):
            # Resident rhs: both K-chunks of 2*realT shard, in per-i-tile
            # slices so the first matmul group starts early.
            rhs_sb = constp.tile([128, 2 * SHARD], bf16)
            nc.sync.dma_start(out=rhs_sb[:, 0:NT], in_=rhs[0:128, 0:NT])
            nc.sync.dma_start(out=rhs_sb[:, SHARD:SHARD + NT],
                              in_=rhs[128:256, 0:NT])
            rhs8_sb = constp.tile([128, 2 * SHARD], fp8)
            nc.sync.dma_start(out=rhs8_sb[:, 0:NT], in_=rhs8[:, 0:NT])
            nc.sync.dma_start(out=rhs8_sb[:, SHARD:SHARD + NT],
                              in_=rhs8[:, SHARD:SHARD + NT])
            # aug rhs rows: 0,1 = -hi/lo(|r_i|^2); 2,3 = 1.0; rest zero,
            # fully materialized host-side (zero-padded to K=128: a K<128
            # matmul stalls the PE pipeline; a device-side memset would
            # serialize the first aug matmul behind Pool)
            aug_sb = constp.tile([128, SHARD], bf16)
            nc.sync.dma_start(out=aug_sb[:, 0:NT], in_=aug[:, 0:NT])
            ones_sb = constp.tile([JTILE, JTILE], bf16)
            nc.sync.dma_start(out=ones_sb[:, :], in_=ones[:, :])

            realv = outp.tile([128, FJT * 8], f32)
            genv = outp.tile([128, NJT * NBLK], f32)

            for jt in range(NJT):
                jo = jt * JTILE
                # spread the 60 real tiles evenly over the 96 slots so the
                # DVE load per slot is uniform
                do_real = (jt * RJT) // NJT != ((jt + 1) * RJT) // NJT
                rjt = (jt * RJT) // NJT
                jor = rjt * JTILE
                lhs_g = lhsp.tile([128, 2 * JTILE], fp8, tag="lhs_g")
                nc.sync.dma_start(
                    out=lhs_g[:, :],
                    in_=colg8[:, jt * 2 * JTILE:(jt + 1) * 2 * JTILE],
                )
                if jt == 0:
                    # remaining const slices, behind jt0's critical loads
                    for it0 in range(1, NIT):
                        io0 = it0 * NT
                        nc.sync.dma_start(out=rhs_sb[:, io0:io0 + NT],
                                          in_=rhs[0:128, io0:io0 + NT])
                        nc.sync.dma_start(
                            out=rhs_sb[:, SHARD + io0:SHARD + io0 + NT],
                            in_=rhs[128:256, io0:io0 + NT])
                        nc.sync.dma_start(out=rhs8_sb[:, io0:io0 + NT],
                                          in_=rhs8[:, io0:io0 + NT])
                        nc.sync.dma_start(
                            out=rhs8_sb[:, SHARD + io0:SHARD + io0 + NT],
                            in_=rhs8[:, SHARD + io0:SHARD + io0 + NT])
                        nc.sync.dma_start(out=aug_sb[:, io0:io0 + NT],
                                          in_=aug[:, io0:io0 + NT])
                if do_real:
                    lhs_r = lhsp.tile([128, 2 * JTILE], bf16, tag="lhs_r")
                    nc.sync.dma_start(
                        out=lhs_r[:, :].rearrange("p (c j) -> p c j", c=2),
                        in_=colr[:, jor:jor + JTILE].rearrange(
                            "(c p) j -> p c j", c=2),
                    )
                    auglr_t = lhsp.tile([128, JTILE], bf16, tag="auglr_t")
                    nc.sync.dma_start(out=auglr_t[:, :],
                                      in_=auglr[:, jor:jor + JTILE])
                    wide_r = widep.tile([128, SHARD], f32, tag="wide_r")

                # gen: one 3-bank PSUM tile, reduced PSUM-direct by the DVE
                ps_g = psgp.tile([128, SHARD], f32, tag="ps_g")
                for it in range(NIT):
                    io = it * NT
                    nc.tensor.matmul(
                        out=ps_g[:, io:io + NT],
                        lhsT=lhs_g[:, :].rearrange("p (t j) -> p t j", t=2),
                        rhs=rhs8_sb[:, :].rearrange(
                            "p (t i) -> p t i", t=2)[:, :, io:io + NT],
                        start=True, stop=False,
                        perf_mode=mybir.MatmulPerfMode.DoubleRow,
                    )
                    nc.tensor.matmul(
                        out=ps_g[:, io:io + NT],
                        lhsT=ones_sb[:, :],
                        rhs=aug_sb[:, io:io + NT],
                        start=False, stop=True,
                    )
                    if do_real:
                        io2 = io
                        ps_r = psrp.tile([128, NT], f32, tag="ps_r")
                        nc.tensor.matmul(
                            out=ps_r[:, :],
                            lhsT=lhs_r[:, 0:JTILE],
                            rhs=rhs_sb[:, io2:io2 + NT],
                            start=True, stop=False,
                        )
                        nc.tensor.matmul(
                            out=ps_r[:, :],
                            lhsT=lhs_r[:, JTILE:2 * JTILE],
                            rhs=rhs_sb[:, SHARD + io2:SHARD + io2 + NT],
                            start=False, stop=False,
                        )
                        nc.tensor.matmul(
                            out=ps_r[:, :],
                            lhsT=auglr_t[:, :],
                            rhs=aug_sb[:, io2:io2 + NT],
                            start=False, stop=True,
                        )
                        nc.scalar.activation(
                            out=wide_r[:, io2:io2 + NT],
                            in_=ps_r[:, :],
                            func=mybir.ActivationFunctionType.Copy,
                        )

                # per-128-block maxima of X, straight out of PSUM
                nc.vector.tensor_reduce(
                    out=genv[:, jt * NBLK:(jt + 1) * NBLK],
                    in_=ps_g[:, :].rearrange("p (b x) -> p b x", b=NBLK),
                    axis=mybir.AxisListType.X,
                    op=mybir.AluOpType.max)

                if do_real:
                    if rjt < 12:
                        # self block: top-8 (host drops the diagonal top-1)
                        nc.vector.max(out=realv[:, rjt * 8:(rjt + 1) * 8],
                                      in_=wide_r[:, :])
                    elif rjt < FJT:
                        # only the max is needed: plain reduce is cheaper
                        nc.vector.tensor_reduce(
                            out=realv[:, rjt * 8:rjt * 8 + 1],
                            in_=wide_r[:, :],
                            axis=mybir.AxisListType.X,
                            op=mybir.AluOpType.max)
                    if PAR_LO <= rjt < PAR_HI:
                        par_t = parp.tile([128, SHARD], f32, tag="par_t")
                        nc.gpsimd.partition_all_reduce(
                            par_t[:, :], wide_r[:, :], channels=128,
                            reduce_op=bass_isa.ReduceOp.max)
                        nc.sync.dma_start(
                            out=o_par[rjt - PAR_LO:rjt - PAR_LO + 1, :],
                            in_=par_t[0:1, :])

            nc.sync.dma_start(out=o_realv[:, :], in_=realv[:, :])
            nc.sync.dma_start(out=o_genv[:, :], in_=genv[:, :])

    nc.compile()
    return nc


def _hilo(x, bf):
    hi = x.astype(bf)
    lo = (x - hi.astype(np.float32)).astype(bf)
    return hi, lo


def kernel(real_stats, gen_stats, _trace=False):
    import ml_dtypes
    from concourse.bass_utils import run_bass_kernel_spmd

    bf = ml_dtypes.bfloat16
    global _cached_nc
    real = np.ascontiguousarray(np.asarray(real_stats, dtype=np.float32))
    gen = np.ascontiguousarray(np.asarray(gen_stats, dtype=np.float32))

    f8 = ml_dtypes.float8_e4m3
    realT = np.ascontiguousarray(real.T)                  # [D, N]
    genT = np.ascontiguousarray(gen.T)
    rhs_bf = (2.0 * realT).astype(bf)                     # [D, N]
    rhs_f32 = 2.0 * realT                                 # [D, N] f32
    # fp8 DoubleRow gen columns: [p, (jt, t, j)] = genT[t*128+p, jt*128+j]
    colg8_np = np.ascontiguousarray(
        genT.reshape(2, 128, NJT, JTILE).transpose(1, 2, 0, 3)
        .reshape(128, NJT * 2 * JTILE)).astype(f8)
    b2 = np.sum(real.astype(np.float64) ** 2, axis=1).astype(np.float32)
    a2g = np.sum(gen.astype(np.float64) ** 2, axis=1).astype(np.float32)
    ones = np.zeros((JTILE, JTILE), dtype=bf)
    ones[0:2, :] = 1

    RW = RJT * JTILE                                      # 7680 rotated cols
    in_maps = []
    for c in range(NCORES):
        sl = slice(c * SHARD, (c + 1) * SHARD)
        negb2_hi, negb2_lo = _hilo(-b2[sl], bf)
        aug4 = np.zeros((128, SHARD), dtype=bf)
        aug4[0] = negb2_hi
        aug4[1] = negb2_lo
        aug4[2:4] = 1
        colr_rot = np.roll(realT, -c * SHARD, axis=1)[:, :RW]
        a2rot = np.roll(b2, -c * SHARD)[:RW]
        nega2_hi, nega2_lo = _hilo(-a2rot, bf)
        auglr = np.zeros((JTILE, RW), dtype=bf)
        auglr[0:2] = 1
        auglr[2] = nega2_hi
        auglr[3] = nega2_lo
        rhs8_np = np.ascontiguousarray(
            rhs_f32[:, sl].reshape(2, 128, SHARD).transpose(1, 0, 2)
            .reshape(128, 2 * SHARD)).astype(f8)
        in_maps.append({
            "colr": colr_rot.astype(bf),
            "colg8": colg8_np,
            "auglr": auglr,
            "rhs": np.ascontiguousarray(rhs_bf[:, sl]),
            "rhs8": rhs8_np,
            "aug": aug4,
            "ones": ones,
        })

    if _cached_nc is None:
        _cached_nc = _build_nc()
    res = run_bass_kernel_spmd(_cached_nc, in_maps,
                               core_ids=list(range(NCORES)),
                               trace=_trace)

    # ---- host combine ----
    # real: Y = -dist^2 candidates, min-combined over all sources
    cand = np.full(N, np.inf, dtype=np.float64)
    p_idx = np.arange(128)
    for c in range(NCORES):
        rv = res.results[c]["o_realv"].reshape(128, FJT, 8)
        top1 = rv[:, :, 0]
        top2 = rv[:, :, 1]
        # self block (k < 12) contains the diagonal: Y_diag ~ 0, true
        # neighbors ~ -300 -> take top2 there when top1 is diag-like
        use2 = np.zeros((128, FJT), dtype=bool)
        use2[:, :12] = top1[:, :12] > -10.0
        y = np.where(use2, top2, top1)                    # [128, FJT]
        jglob = (c * SHARD + np.arange(FJT)[None, :] * JTILE
                 + p_idx[:, None]) % N
        np.minimum.at(cand, jglob.ravel(), (-y).ravel())
        par = res.results[c]["o_par"]                     # [48, SHARD] of Y
        par_min = -par.max(axis=0)                        # min dist^2 per i
        sl = slice(c * SHARD, (c + 1) * SHARD)
        cand[sl] = np.minimum(cand[sl], par_min)
    realNN = np.sqrt(np.maximum(cand, 0.0))               # [N]

    # gen: X = 2G - |r_i|^2 block maxima -> host argmin refinement.
    # Xb[g, j]: per-gen-column max of X over global 128-block g.
    Xb = np.empty((NCORES * NBLK, N), dtype=np.float32)
    for c in range(NCORES):
        gv = res.results[c]["o_genv"].reshape(128, NJT, NBLK)
        # j = jt*128 + p ; global block = c*NBLK + b
        Xb[c * NBLK:(c + 1) * NBLK, :] = (
            gv.transpose(2, 1, 0).reshape(NBLK, N))
    best = Xb.max(axis=0)                                 # [N] max X
    # refine every block whose coarse max is within MARGIN of the best
    cand_mask = Xb >= (best - MARGIN)[None, :]            # [96, N]
    Xstar = np.full(N, -np.inf, dtype=np.float32)
    istar = np.zeros(N, dtype=np.int64)
    genf = gen.astype(np.float32)
    for g in range(NCORES * NBLK):
        js = np.nonzero(cand_mask[g])[0]
        if js.size == 0:
            continue
        rb = real[g * JTILE:(g + 1) * JTILE]              # [128, D]
        Xex = 2.0 * (genf[js] @ rb.T) - b2[g * JTILE:(g + 1) * JTILE][None, :]
        loc = np.argmax(Xex, axis=1)
        val = Xex[np.arange(js.size), loc]
        upd = val > Xstar[js]
        Xstar[js[upd]] = val[upd]
        istar[js[upd]] = g * JTILE + loc[upd]
    d1 = np.sqrt(np.maximum(a2g - Xstar, 0.0))
    d2 = realNN[istar]

    z = (d2 - d1) / 0.1
    authen = np.where(z >= 0, 1.0 / (1.0 + np.exp(-np.abs(z))),
                      np.exp(-np.abs(z)) / (1.0 + np.exp(-np.abs(z))))
    out = np.asarray(-100.0 * np.mean(authen), dtype=np.float32)
    if _trace:
        return out, res
    return out


# revision 10
# speedup vs baseline: 1.2981x; 1.0334x over previous
"""AuthPct metric kernel for 8 Trainium2 NeuronCores.

Sharding: real_stats rows are sharded across the 8 cores (1536 each).
Each core computes coarse (fp8) negated squared distances

    Y'[j, i] = 2*f_j[0:254].r_i[0:254] - |r_i|^2          (gen tiles)
    W[q, i]  = Y'[q, i] - |r_q|^2  ~=  -dist^2(r_q, r_i)  (real tiles)

with ONE fp8 DoubleRow matmul (K=256) per 512-i PSUM bank: k-rows
0..253 carry the features, k-rows 254/255 carry a scaled hi/lo fp8
split of -|r_i|^2 (lhsT rows 254/255 are the matching scale).  No
augmented matmul and no bf16 pass: the PE does 3 matmuls per
[128 x 1536] tile.  The per-partition -|r_q|^2 of real tiles is added
exactly by the ScalarE Identity-activation bias during PSUM->SBUF
evacuation.

Reductions produce per-128-block coarse partials only:
 - gen: DVE segmented tensor_reduce (max) straight out of PSUM ->
   o_genv [128, 96*12], no ScalarE copy.
 - real j-side: DVE segmented reduce (max of -d^2) on evacuated tiles
   m=0..3 -> o_realv [128, 48*12]; the self-tile diagonal lands in
   block b==rjt and is masked on the host.
 - real i-side: Pool partition_all_reduce(max) on tiles m=1..4 ->
   o_par [48, 1536] (q-tile block resolution).

The host min-combines the coarse partials, then refines exactly (tiny
f32 gemms over 128-real blocks within a noise margin of each coarse
winner) to recover the exact gen argmin + d1 and the exact realNN
values at the used indices.  The fp8/254-dim noise only widens the
refinement margin; the final values are exact fp32.
"""

import numpy as np

N = 12288
D = 256
DE = 254                     # feature dims carried on device
NCORES = 8
SHARD = N // NCORES          # 1536 rows per core
JTILE = 128                  # j columns per tile (PSUM partitions)
NJT = N // JTILE             # 96 gen j-tiles
RJT = 60                     # real j-tiles: shards c..c+4 (rotated)
FJT = 48                     # real j-tiles with a DVE free-side scan (m=0..3)
PAR_LO, PAR_HI = 12, 60      # real j-tiles with partition-reduce harvest
NT = 512                     # i elements per matmul (PSUM bank)
NIT = SHARD // NT            # 3 i-tiles
NBLK = SHARD // JTILE        # 12 i-blocks of 128 per core
NSCALE = 4.0                 # fp8 norm-row scale: rows carry -|r|^2/NSCALE
MARGIN_G = 12.0              # d^2 margin for gen argmin refinement
MARGIN_R = 12.0              # d^2 margin for realNN refinement

_cached_nc = None


def _build_nc():
    import concourse.bass_isa as bass_isa
    import concourse.mybir as mybir
    from concourse import bacc
    from concourse.tile import TileContext

    f32 = mybir.dt.float32
    fp8 = mybir.dt.float8e4

    nc = bacc.Bacc("TRN2", target_bir_lowering=False, debug=False,
                   num_devices=NCORES)

    # DoubleRow layouts: [p, (tile, t, col)] with K row = t*128 + p
    colg8 = nc.dram_tensor("colg8", [128, NJT * 2 * JTILE], fp8,
                           kind="ExternalInput")
    colr8 = nc.dram_tensor("colr8", [128, RJT * 2 * JTILE], fp8,
                           kind="ExternalInput")
    rhs8 = nc.dram_tensor("rhs8", [128, 2 * SHARD], fp8,
                          kind="ExternalInput")
    negb2r = nc.dram_tensor("negb2r", [128, RJT], f32,
                            kind="ExternalInput")

    o_realv = nc.dram_tensor("o_realv", [128, FJT * NBLK], f32,
                             kind="ExternalOutput")
    o_par = nc.dram_tensor("o_par", [PAR_HI - PAR_LO, SHARD], f32,
                           kind="ExternalOutput")
    o_genv = nc.dram_tensor("o_genv", [128, NJT * NBLK], f32,
                            kind="ExternalOutput")

    with TileContext(nc) as tc:
        with (
            tc.tile_pool(name="const", bufs=1) as constp,
            tc.tile_pool(name="lhs", bufs=6) as lhsp,
            tc.tile_pool(name="wide", bufs=4) as widep,
            tc.tile_pool(name="parp", bufs=3) as parp,
            tc.tile_pool(name="outb", bufs=1) as outp,
            tc.tile_pool(name="psg", bufs=2, space="PSUM") as psgp,
            tc.tile_pool(name="psr", bufs=2, space="PSUM") as psrp,
        ):
            rhs8_sb = constp.tile([128, 2 * SHARD], fp8)
            nc.sync.dma_start(out=rhs8_sb[:, 0:NT], in_=rhs8[:, 0:NT])
            nc.sync.dma_start(out=rhs8_sb[:, SHARD:SHARD + NT],
                              in_=rhs8[:, SHARD:SHARD + NT])
            negb2_sb = constp.tile([128, RJT], f32)
            nc.sync.dma_start(out=negb2_sb[:, :], in_=negb2r[:, :])

            realv = outp.tile([128, FJT * NBLK], f32)
            genv = outp.tile([128, NJT * NBLK], f32)

            def rhs_ap(io):
                return rhs8_sb[:, :].rearrange(
                    "p (t i) -> p t i", t=2)[:, :, io:io + NT]

            for jt in range(NJT):
                do_real = (jt * RJT) // NJT != ((jt + 1) * RJT) // NJT
                rjt = (jt * RJT) // NJT
                lhs_g = lhsp.tile([128, 2 * JTILE], fp8, tag="lhs_g")
                nc.sync.dma_start(
                    out=lhs_g[:, :],
                    in_=colg8[:, jt * 2 * JTILE:(jt + 1) * 2 * JTILE],
                )
                if jt == 0:
                    # remaining const slices, behind jt0's critical loads
                    for it0 in range(1, NIT):
                        io0 = it0 * NT
                        nc.sync.dma_start(out=rhs8_sb[:, io0:io0 + NT],
                                          in_=rhs8[:, io0:io0 + NT])
                        nc.sync.dma_start(
                            out=rhs8_sb[:, SHARD + io0:SHARD + io0 + NT],
                            in_=rhs8[:, SHARD + io0:SHARD + io0 + NT])
                if do_real:
                    lhs_r = lhsp.tile([128, 2 * JTILE], fp8, tag="lhs_r")
                    nc.sync.dma_start(
                        out=lhs_r[:, :],
                        in_=colr8[:, rjt * 2 * JTILE:(rjt + 1) * 2 * JTILE],
                    )
                    wide_r = widep.tile([128, SHARD], f32, tag="wide_r")

                # gen: one 3-bank PSUM tile, reduced PSUM-direct by the DVE
                ps_g = psgp.tile([128, SHARD], f32, tag="ps_g")
                for it in range(NIT):
                    io = it * NT
                    nc.tensor.matmul(
                        out=ps_g[:, io:io + NT],
                        lhsT=lhs_g[:, :].rearrange("p (t j) -> p t j", t=2),
                        rhs=rhs_ap(io),
                        start=True, stop=True,
                        perf_mode=mybir.MatmulPerfMode.DoubleRow,
                    )
                    if do_real:
                        ps_r = psrp.tile([128, NT], f32, tag="ps_r")
                        nc.tensor.matmul(
                            out=ps_r[:, :],
                            lhsT=lhs_r[:, :].rearrange(
                                "p (t j) -> p t j", t=2),
                            rhs=rhs_ap(io),
                            start=True, stop=True,
                            perf_mode=mybir.MatmulPerfMode.DoubleRow,
                        )
                        # W = Y' - |r_q|^2 = -dist^2 (exact f32 bias)
                        nc.scalar.activation(
                            out=wide_r[:, io:io + NT],
                            in_=ps_r[:, :],
                            func=mybir.ActivationFunctionType.Identity,
                            bias=negb2_sb[:, rjt:rjt + 1],
                        )

                # per-128-block maxima of X, straight out of PSUM
                nc.vector.tensor_reduce(
                    out=genv[:, jt * NBLK:(jt + 1) * NBLK],
                    in_=ps_g[:, :].rearrange("p (b x) -> p b x", b=NBLK),
                    axis=mybir.AxisListType.X,
                    op=mybir.AluOpType.max)

                if do_real:
                    if rjt < FJT:
                        nc.vector.tensor_reduce(
                            out=realv[:, rjt * NBLK:(rjt + 1) * NBLK],
                            in_=wide_r[:, :].rearrange(
                                "p (b x) -> p b x", b=NBLK),
                            axis=mybir.AxisListType.X,
                            op=mybir.AluOpType.max)
                    if PAR_LO <= rjt < PAR_HI:
                        par_t = parp.tile([128, SHARD], f32, tag="par_t")
                        nc.gpsimd.partition_all_reduce(
                            par_t[:, :], wide_r[:, :], channels=128,
                            reduce_op=bass_isa.ReduceOp.max)
                        nc.sync.dma_start(
                            out=o_par[rjt - PAR_LO:rjt - PAR_LO + 1, :],
                            in_=par_t[0:1, :])

            nc.sync.dma_start(out=o_realv[:, :], in_=realv[:, :])
            nc.sync.dma_start(out=o_genv[:, :], in_=genv[:, :])

    nc.compile()
    return nc


def _norm_rows8(b2v, f8):
    """-b2/NSCALE as hi/lo fp8 rows (device multiplies back by NSCALE)."""
    t = -b2v / NSCALE
    hi = t.astype(f8)
    lo = (t - hi.astype(np.float32)).astype(f8)
    return hi, lo


def _dr_pack(featT, f8, norm_hi, norm_lo):
    """[256-K, C] f32 -> DoubleRow fp8 [128, C*2... ] per 128-col tiles.

    featT rows 0..253 are features; rows 254/255 are replaced by the
    scaled norm hi/lo (rhs) or the NSCALE constant (lhs side).
    Returns [128, ntiles*2*128] with layout (p, tile, t, col).
    """
    Dd, C = featT.shape
    assert Dd == D and C % JTILE == 0
    nt_ = C // JTILE
    a = featT.copy()
    a[DE] = norm_hi.astype(np.float32) if norm_hi is not None else NSCALE
    a[DE + 1] = norm_lo.astype(np.float32) if norm_lo is not None else NSCALE
    # [t*128+p, tile*128+j] -> [p, tile, t, j]
    out = (a.reshape(2, 128, nt_, JTILE).transpose(1, 2, 0, 3)
           .reshape(128, nt_ * 2 * JTILE))
    return np.ascontiguousarray(out).astype(f8)


def kernel(real_stats, gen_stats, _trace=False):
    import ml_dtypes
    from concourse.bass_utils import run_bass_kernel_spmd

    f8 = ml_dtypes.float8_e4m3
    global _cached_nc
    real = np.ascontiguousarray(np.asarray(real_stats, dtype=np.float32))
    gen = np.ascontiguousarray(np.asarray(gen_stats, dtype=np.float32))

    realT = np.ascontiguousarray(real.T)                  # [D, N]
    genT = np.ascontiguousarray(gen.T)
    b2 = np.sum(real.astype(np.float64) ** 2, axis=1).astype(np.float32)
    a2g = np.sum(gen.astype(np.float64) ** 2, axis=1).astype(np.float32)

    colg8_np = _dr_pack(genT, f8, None, None)             # lhs: scale rows

    RW = RJT * JTILE                                      # 7680 rotated cols
    in_maps = []
    for c in range(NCORES):
        sl = slice(c * SHARD, (c + 1) * SHARD)
        hi, lo = _norm_rows8(b2[sl], f8)
        rhs_full = 2.0 * realT[:, sl]
        rhs_full[DE] = hi.astype(np.float32)
        rhs_full[DE + 1] = lo.astype(np.float32)
        rhs8_np = np.ascontiguousarray(
            rhs_full.reshape(2, 128, SHARD).transpose(1, 0, 2)
            .reshape(128, 2 * SHARD)).astype(f8)
        colr_rot = np.roll(realT, -c * SHARD, axis=1)[:, :RW]
        colr8_np = _dr_pack(colr_rot, f8, None, None)
        b2rot = np.roll(b2, -c * SHARD)[:RW]
        negb2_np = np.ascontiguousarray(
            -b2rot.reshape(RJT, 128).T)                   # [128, RJT]
        in_maps.append({
            "colg8": colg8_np,
            "colr8": colr8_np,
            "rhs8": rhs8_np,
            "negb2r": negb2_np,
        })

    if _cached_nc is None:
        _cached_nc = _build_nc()
    res = run_bass_kernel_spmd(_cached_nc, in_maps,
                               core_ids=list(range(NCORES)),
                               trace=_trace)

    # ---- host combine ----
    NB = NCORES * NBLK                                    # 96 global blocks
    # real: coarse min d^2 per (real, block) from both scan directions
    d2blk = np.full((N, NB), np.inf, dtype=np.float32)
    for c in range(NCORES):
        rv = res.results[c]["o_realv"].reshape(128, FJT, NBLK)
        # q = (c*SHARD + rjt*128 + p) % N ; candidate block = c*NBLK + b
        q = (c * SHARD + np.arange(FJT)[None, :, None] * JTILE
             + np.arange(128)[:, None, None]) % N         # [128, FJT, 1]
        gb = c * NBLK + np.arange(NBLK)[None, None, :]    # [1, 1, NBLK]
        idx = (q * NB + gb).ravel()
        np.minimum.at(d2blk.ravel(), idx, (-rv).ravel())
        par = res.results[c]["o_par"]                     # [48, SHARD]
        i = c * SHARD + np.arange(SHARD)[None, :]         # [1, SHARD]
        qb = ((c * NBLK + np.arange(PAR_LO, PAR_HI)[:, None]) % NB)
        idx2 = (i * NB + qb).ravel()
        np.minimum.at(d2blk.ravel(), idx2, (-par).ravel())
    # the self-tile diagonal contaminates block r//128: mask it
    d2blk[np.arange(N), np.arange(N) // JTILE] = np.inf

    # gen: coarse block maxima of X = 2g.r - |r|^2
    Xb = np.empty((NB, N), dtype=np.float32)
    for c in range(NCORES):
        gv = res.results[c]["o_genv"].reshape(128, NJT, NBLK)
        Xb[c * NBLK:(c + 1) * NBLK, :] = (
            gv.transpose(2, 1, 0).reshape(NBLK, N))
    best = Xb.max(axis=0)
    cand_mask = Xb >= (best - MARGIN_G)[None, :]          # [96, N]
    Xstar = np.full(N, -np.inf, dtype=np.float32)
    istar = np.zeros(N, dtype=np.int64)
    for g in range(NB):
        js = np.nonzero(cand_mask[g])[0]
        if js.size == 0:
            continue
        rb = real[g * JTILE:(g + 1) * JTILE]              # [128, D]
        Xex = 2.0 * (gen[js] @ rb.T) - b2[g * JTILE:(g + 1) * JTILE][None, :]
        loc = np.argmax(Xex, axis=1)
        val = Xex[np.arange(js.size), loc]
        upd = val > Xstar[js]
        Xstar[js[upd]] = val[upd]
        istar[js[upd]] = g * JTILE + loc[upd]
    d1 = np.sqrt(np.maximum(a2g - Xstar, 0.0))

    # realNN: exact refinement only at the used indices
    used = np.unique(istar)
    du = d2blk[used]                                      # [U, 96]
    coarse = du.min(axis=1)
    rcand = du <= (coarse + MARGIN_R)[:, None]
    rcand[np.arange(used.size), used // JTILE] = True     # always refine diag
    nn2 = np.full(used.size, np.inf, dtype=np.float32)
    for g in range(NB):
        rs = np.nonzero(rcand[:, g])[0]
        if rs.size == 0:
            continue
        ridx = used[rs]
        rb = real[g * JTILE:(g + 1) * JTILE]
        d2 = (b2[ridx][:, None] + b2[g * JTILE:(g + 1) * JTILE][None, :]
              - 2.0 * (real[ridx] @ rb.T))
        inblk = (ridx >= g * JTILE) & (ridx < (g + 1) * JTILE)
        d2[inblk, ridx[inblk] - g * JTILE] = np.inf       # exclude self
        nn2[rs] = np.minimum(nn2[rs], d2.min(axis=1))
    realNN_used = np.sqrt(np.maximum(nn2, 0.0))
    lut = np.zeros(N, dtype=np.float32)
    lut[used] = realNN_used
    d2v = lut[istar]

    z = (d2v - d1) / 0.1
    authen = np.where(z >= 0, 1.0 / (1.0 + np.exp(-np.abs(z))),
                      np.exp(-np.abs(z)) / (1.0 + np.exp(-np.abs(z))))
    out = np.asarray(-100.0 * np.mean(authen), dtype=np.float32)
    if _trace:
        return out, res
    return out


# revision 11
# speedup vs baseline: 1.7629x; 1.3581x over previous
"""AuthPct metric kernel for 8 Trainium2 NeuronCores.

Sharding: real_stats rows are sharded across the 8 cores (1536 each,
the i/rhs side); gen and (host-rotated) real columns are the lhs side.
Each core computes coarse (fp8) tiles

    X[j, i] = 2*f_j[0:254].r_i[0:254] - |r_i|^2     [j-tile, 1536 i]

with ONE fp8 DoubleRow matmul (K=256) per 512-i PSUM bank: k-rows
0..253 carry features, k-rows 254/255 carry a scaled hi/lo fp8 split
of -|r_i|^2.  No augmented matmul, no bf16 pass, no PSUM->SBUF copy.

Per j-slot the core processes one gen j-tile and one real j-tile (the
real side uses the full 96-tile rotation, so every ordered real pair
appears once as a row-perspective; no partition reduce is needed):

 - gen: DVE segmented tensor_reduce (max) straight out of the 3-bank
   PSUM tile -> per-128-block coarse maxima, o_genv [128, 96*12].
 - real: ScalarE smooth-min straight out of each PSUM bank:
   ACTIVATE Exp with scale 1/T, per-partition bias (C0-|r_q|^2)/T and
   accum_out -> acc = sum_i exp((C0 - d^2(q,i))/T), one f32 per
   (row q, 512-block), o_reals [128, 96*3].  -T*ln(acc)+C0 is a
   smooth lower bound of the block min d^2 (within T*ln(n_eff)).

The Pool engine (5.4us per partition-reduce, the old co-bottleneck) is
not used at all.

The host min-combines the coarse partials, then refines exactly (f32
gemms over candidate blocks within a noise margin of each coarse
winner; the real diagonal block is always refined) to recover the
exact gen argmin + d1 and exact realNN at the used indices.  Device
noise (fp8, dropped dims, smooth-min slack) only widens the margins;
the returned values are exact fp32.
"""

import numpy as np

N = 12288
D = 256
DE = 254                     # feature dims carried on device
NCORES = 8
SHARD = N // NCORES          # 1536 rows per core
JTILE = 128                  # j columns per tile (PSUM partitions)
NJT = N // JTILE             # 96 gen j-tiles
RJT = 96                     # real j-tiles: full rotation c..c+7
NT = 512                     # i elements per matmul (PSUM bank)
NIT = SHARD // NT            # 3 i-tiles
NBLK = SHARD // JTILE        # 12 i-blocks of 128 per core
NSB = N // NT                # 24 global 512-blocks
NSCALE = 4.0                 # fp8 norm-row scale: rows carry -|r|^2/NSCALE
MARGIN_G = 12.0              # d^2 margin for gen argmin refinement
MARGIN_R = 20.0              # d^2 margin for realNN refinement
TSM = 4.0                    # smooth-min temperature
C0 = 250.0                   # smooth-min shift (~min real-real NN d^2)

_cached_nc = None


def _build_nc():
    import concourse.mybir as mybir
    from concourse import bacc
    from concourse.tile import TileContext

    f32 = mybir.dt.float32
    fp8 = mybir.dt.float8e4

    nc = bacc.Bacc("TRN2", target_bir_lowering=False, debug=False,
                   num_devices=NCORES)

    # DoubleRow layouts: [p, (tile, t, col)] with K row = t*128 + p
    colg8 = nc.dram_tensor("colg8", [128, NJT * 2 * JTILE], fp8,
                           kind="ExternalInput")
    colr8 = nc.dram_tensor("colr8", [128, RJT * 2 * JTILE], fp8,
                           kind="ExternalInput")
    rhs8 = nc.dram_tensor("rhs8", [128, 2 * SHARD], fp8,
                          kind="ExternalInput")
    biasr = nc.dram_tensor("biasr", [128, RJT], f32,
                           kind="ExternalInput")

    o_genv = nc.dram_tensor("o_genv", [128, NJT * NBLK], f32,
                            kind="ExternalOutput")
    o_reals = nc.dram_tensor("o_reals", [128, RJT * NIT], f32,
                             kind="ExternalOutput")

    with TileContext(nc) as tc:
        with (
            tc.tile_pool(name="const", bufs=1) as constp,
            tc.tile_pool(name="lhs", bufs=6) as lhsp,
            tc.tile_pool(name="junk", bufs=3) as junkp,
            tc.tile_pool(name="outb", bufs=1) as outp,
            tc.tile_pool(name="psg", bufs=2, space="PSUM") as psgp,
            tc.tile_pool(name="psr", bufs=2, space="PSUM") as psrp,
        ):
            rhs8_sb = constp.tile([128, 2 * SHARD], fp8)
            nc.sync.dma_start(out=rhs8_sb[:, 0:NT], in_=rhs8[:, 0:NT])
            nc.sync.dma_start(out=rhs8_sb[:, SHARD:SHARD + NT],
                              in_=rhs8[:, SHARD:SHARD + NT])
            biasr_sb = constp.tile([128, RJT], f32)
            nc.sync.dma_start(out=biasr_sb[:, :], in_=biasr[:, :])

            genv = outp.tile([128, NJT * NBLK], f32)
            racc = outp.tile([128, RJT * NIT], f32)

            def rhs_ap(io):
                return rhs8_sb[:, :].rearrange(
                    "p (t i) -> p t i", t=2)[:, :, io:io + NT]

            for jt in range(NJT):
                lhs_g = lhsp.tile([128, 2 * JTILE], fp8, tag="lhs_g")
                nc.sync.dma_start(
                    out=lhs_g[:, :],
                    in_=colg8[:, jt * 2 * JTILE:(jt + 1) * 2 * JTILE],
                )
                lhs_r = lhsp.tile([128, 2 * JTILE], fp8, tag="lhs_r")
                nc.sync.dma_start(
                    out=lhs_r[:, :],
                    in_=colr8[:, jt * 2 * JTILE:(jt + 1) * 2 * JTILE],
                )
                if jt == 0:
                    # remaining const slices, behind jt0's critical loads
                    for it0 in range(1, NIT):
                        io0 = it0 * NT
                        nc.sync.dma_start(out=rhs8_sb[:, io0:io0 + NT],
                                          in_=rhs8[:, io0:io0 + NT])
                        nc.sync.dma_start(
                            out=rhs8_sb[:, SHARD + io0:SHARD + io0 + NT],
                            in_=rhs8[:, SHARD + io0:SHARD + io0 + NT])

                # gen: one 3-bank PSUM tile, reduced PSUM-direct by the DVE
                ps_g = psgp.tile([128, SHARD], f32, tag="ps_g")
                for it in range(NIT):
                    io = it * NT
                    nc.tensor.matmul(
                        out=ps_g[:, io:io + NT],
                        lhsT=lhs_g[:, :].rearrange("p (t j) -> p t j", t=2),
                        rhs=rhs_ap(io),
                        start=True, stop=True,
                        perf_mode=mybir.MatmulPerfMode.DoubleRow,
                    )
                    ps_r = psrp.tile([128, NT], f32, tag="ps_r")
                    nc.tensor.matmul(
                        out=ps_r[:, :],
                        lhsT=lhs_r[:, :].rearrange("p (t j) -> p t j", t=2),
                        rhs=rhs_ap(io),
                        start=True, stop=True,
                        perf_mode=mybir.MatmulPerfMode.DoubleRow,
                    )
                    # acc[q] = sum_i exp((X - |r_q|^2 + C0)/T), PSUM-direct
                    junk = junkp.tile([128, NT], f32, tag="junk")
                    nc.scalar.activation(
                        out=junk[:, :],
                        in_=ps_r[:, :],
                        func=mybir.ActivationFunctionType.Exp,
                        bias=biasr_sb[:, jt:jt + 1],
                        scale=1.0 / TSM,
                        accum_out=racc[:, jt * NIT + it:jt * NIT + it + 1],
                    )

                # per-128-block maxima of X, straight out of PSUM
                nc.vector.tensor_reduce(
                    out=genv[:, jt * NBLK:(jt + 1) * NBLK],
                    in_=ps_g[:, :].rearrange("p (b x) -> p b x", b=NBLK),
                    axis=mybir.AxisListType.X,
                    op=mybir.AluOpType.max)

            nc.sync.dma_start(out=o_genv[:, :], in_=genv[:, :])
            nc.sync.dma_start(out=o_reals[:, :], in_=racc[:, :])

    nc.compile()
    return nc


def _dr_pack(featT, f8, norm_hi, norm_lo):
    """[256-K, C] f32 -> fp8 DoubleRow [128, (tile, t, col)] layout.

    Rows 254/255 get the scaled norm hi/lo (rhs side) or the NSCALE
    constant (lhs side).
    """
    Dd, C = featT.shape
    assert Dd == D and C % JTILE == 0
    nt_ = C // JTILE
    a = featT.copy()
    a[DE] = norm_hi if norm_hi is not None else NSCALE
    a[DE + 1] = norm_lo if norm_lo is not None else NSCALE
    out = (a.reshape(2, 128, nt_, JTILE).transpose(1, 2, 0, 3)
           .reshape(128, nt_ * 2 * JTILE))
    return np.ascontiguousarray(out).astype(f8)


def kernel(real_stats, gen_stats, _trace=False):
    import ml_dtypes
    from concourse.bass_utils import run_bass_kernel_spmd

    f8 = ml_dtypes.float8_e4m3
    global _cached_nc
    real = np.ascontiguousarray(np.asarray(real_stats, dtype=np.float32))
    gen = np.ascontiguousarray(np.asarray(gen_stats, dtype=np.float32))

    realT = np.ascontiguousarray(real.T)                  # [D, N]
    genT = np.ascontiguousarray(gen.T)
    b2 = np.sum(real.astype(np.float64) ** 2, axis=1).astype(np.float32)
    a2g = np.sum(gen.astype(np.float64) ** 2, axis=1).astype(np.float32)

    colg8_np = _dr_pack(genT, f8, None, None)

    in_maps = []
    for c in range(NCORES):
        sl = slice(c * SHARD, (c + 1) * SHARD)
        t = -b2[sl] / NSCALE
        hi = t.astype(f8)
        lo = (t - hi.astype(np.float32)).astype(f8)
        rhs_full = 2.0 * realT[:, sl]
        rhs_full[DE] = hi.astype(np.float32)
        rhs_full[DE + 1] = lo.astype(np.float32)
        rhs8_np = np.ascontiguousarray(
            rhs_full.reshape(2, 128, SHARD).transpose(1, 0, 2)
            .reshape(128, 2 * SHARD)).astype(f8)
        colr_rot = np.roll(realT, -c * SHARD, axis=1)     # full rotation
        colr8_np = _dr_pack(colr_rot, f8, None, None)
        b2rot = np.roll(b2, -c * SHARD)
        biasr_np = np.ascontiguousarray(
            ((C0 - b2rot) / TSM).reshape(RJT, 128).T)     # [128, RJT]
        in_maps.append({
            "colg8": colg8_np,
            "colr8": colr8_np,
            "rhs8": rhs8_np,
            "biasr": biasr_np.astype(np.float32),
        })

    if _cached_nc is None:
        _cached_nc = _build_nc()
    res = run_bass_kernel_spmd(_cached_nc, in_maps,
                               core_ids=list(range(NCORES)),
                               trace=_trace)

    # ---- host combine ----
    NB = NCORES * NBLK                                    # 96 128-blocks
    # real: smooth-min partials -> coarse d^2 per (real, 512-block)
    d2s = np.full((N, NSB), np.inf, dtype=np.float32)
    for c in range(NCORES):
        acc = res.results[c]["o_reals"].reshape(128, RJT, NIT)
        with np.errstate(divide="ignore"):
            part = C0 - TSM * np.log(acc)                 # [128, RJT, NIT]
        q = (c * SHARD + np.arange(RJT)[None, :, None] * JTILE
             + np.arange(128)[:, None, None]) % N
        sb = c * NIT + np.arange(NIT)[None, None, :]
        idx = (q * NSB + sb).ravel()
        np.minimum.at(d2s.ravel(), idx, part.ravel())
    diag_sb = np.arange(N) // NT
    d2s_m = d2s.copy()
    d2s_m[np.arange(N), diag_sb] = np.inf                 # mask diag block

    # gen: coarse block maxima of X = 2g.r - |r|^2
    Xb = np.empty((NB, N), dtype=np.float32)
    for c in range(NCORES):
        gv = res.results[c]["o_genv"].reshape(128, NJT, NBLK)
        Xb[c * NBLK:(c + 1) * NBLK, :] = (
            gv.transpose(2, 1, 0).reshape(NBLK, N))
    best = Xb.max(axis=0)
    cand_mask = Xb >= (best - MARGIN_G)[None, :]          # [96, N]
    Xstar = np.full(N, -np.inf, dtype=np.float32)
    istar = np.zeros(N, dtype=np.int64)
    for g in range(NB):
        js = np.nonzero(cand_mask[g])[0]
        if js.size == 0:
            continue
        rb = real[g * JTILE:(g + 1) * JTILE]              # [128, D]
        Xex = 2.0 * (gen[js] @ rb.T) - b2[g * JTILE:(g + 1) * JTILE][None, :]
        loc = np.argmax(Xex, axis=1)
        val = Xex[np.arange(js.size), loc]
        upd = val > Xstar[js]
        Xstar[js[upd]] = val[upd]
        istar[js[upd]] = g * JTILE + loc[upd]
    d1 = np.sqrt(np.maximum(a2g - Xstar, 0.0))

    # realNN: exact refinement only at the used indices
    used = np.unique(istar)
    du = d2s_m[used]                                      # [U, 24]
    coarse = du.min(axis=1)
    rcand = du <= (coarse + MARGIN_R)[:, None]
    rcand[~np.isfinite(coarse)] = True                    # fallback: all
    rcand[np.arange(used.size), diag_sb[used]] = True     # always diag
    nn2 = np.full(used.size, np.inf, dtype=np.float32)
    for g in range(NSB):
        rs = np.nonzero(rcand[:, g])[0]
        if rs.size == 0:
            continue
        ridx = used[rs]
        rb = real[g * NT:(g + 1) * NT]
        d2 = (b2[ridx][:, None] + b2[g * NT:(g + 1) * NT][None, :]
              - 2.0 * (real[ridx] @ rb.T))
        inblk = (ridx >= g * NT) & (ridx < (g + 1) * NT)
        d2[inblk, ridx[inblk] - g * NT] = np.inf          # exclude self
        nn2[rs] = np.minimum(nn2[rs], d2.min(axis=1))
    lut = np.zeros(N, dtype=np.float32)
    lut[used] = np.sqrt(np.maximum(nn2, 0.0))
    d2v = lut[istar]

    z = (d2v - d1) / 0.1
    authen = np.where(z >= 0, 1.0 / (1.0 + np.exp(-np.abs(z))),
                      np.exp(-np.abs(z)) / (1.0 + np.exp(-np.abs(z))))
    out = np.asarray(-100.0 * np.mean(authen), dtype=np.float32)
    if _trace:
        return out, res
    return out


# revision 18
# speedup vs baseline: 1.7796x; 1.0095x over previous
"""AuthPct metric kernel for 8 Trainium2 NeuronCores.

Sharding: real_stats rows are sharded across the 8 cores (1536 each,
the i/rhs side); gen and (host-rotated) real columns are the lhs side.
Each core computes coarse (fp8) tiles

    X[j, i] = 2*f_j[0:254].r_i[0:254] - |r_i|^2     [j-tile, 1536 i]

with ONE fp8 DoubleRow matmul (K=256) per 512-i PSUM bank: k-rows
0..253 carry features, k-rows 254/255 carry a scaled hi/lo fp8 split
of -|r_i|^2.  No augmented matmul, no bf16 pass, no PSUM->SBUF copy.

Per j-slot the core processes one gen j-tile and one real j-tile (the
real side uses the full 96-tile rotation, so every ordered real pair
appears once as a row-perspective; no partition reduce is needed):

 - gen: DVE segmented tensor_reduce (max) straight out of the 3-bank
   PSUM tile -> per-128-block coarse maxima, o_genv [128, 96*12].
 - real: ScalarE smooth-min straight out of each PSUM bank:
   ACTIVATE Exp with scale 1/T, per-partition bias (C0-|r_q|^2)/T and
   accum_out -> acc = sum_i exp((C0 - d^2(q,i))/T), one f32 per
   (row q, 512-block), o_reals [128, 96*3].  -T*ln(acc)+C0 is a
   smooth lower bound of the block min d^2 (within T*ln(n_eff)).

The Pool engine (5.4us per partition-reduce, the old co-bottleneck) is
not used at all.

The host min-combines the coarse partials, then refines exactly (f32
gemms over candidate blocks within a noise margin of each coarse
winner; the real diagonal block is always refined) to recover the
exact gen argmin + d1 and exact realNN at the used indices.  Device
noise (fp8, dropped dims, smooth-min slack) only widens the margins;
the returned values are exact fp32.
"""

import numpy as np

N = 12288
D = 256
DE = 254                     # feature dims carried on device
NCORES = 8
SHARD = N // NCORES          # 1536 rows per core
JTILE = 128                  # j columns per tile (PSUM partitions)
NJT = N // JTILE             # 96 gen j-tiles
RJT = 96                     # real j-tiles: full rotation c..c+7
NT = 512                     # i elements per matmul (PSUM bank)
NIT = SHARD // NT            # 3 i-tiles
NBLK = SHARD // JTILE        # 12 i-blocks of 128 per core
NSB = N // NT                # 24 global 512-blocks
NSCALE = 4.0                 # fp8 norm-row scale: rows carry -|r|^2/NSCALE
MARGIN_G = 12.0              # d^2 margin for gen argmin refinement
MARGIN_R = 20.0              # d^2 margin for realNN refinement (smooth)
MARGIN_RV = 12.0             # d^2 margin for realNN refinement (DVE exact)
TSM = 4.0                    # smooth-min temperature
C0 = 250.0                   # smooth-min shift (~min real-real NN d^2)
NVR = 14                     # real tiles scanned by the DVE instead of ScalarE
VSLOT = [jt for jt in range(NJT)
         if (jt * NVR) // NJT != ((jt + 1) * NVR) // NJT]

_cached_nc = None


def _build_nc():
    import concourse.mybir as mybir
    from concourse import bacc
    from concourse.tile import TileContext

    f32 = mybir.dt.float32
    fp8 = mybir.dt.float8e4

    nc = bacc.Bacc("TRN2", target_bir_lowering=False, debug=False,
                   num_devices=NCORES)

    # DoubleRow layouts: [p, (tile, t, col)] with K row = t*128 + p
    colg8 = nc.dram_tensor("colg8", [128, NJT * 2 * JTILE], fp8,
                           kind="ExternalInput")
    colr8 = nc.dram_tensor("colr8", [128, RJT * 2 * JTILE], fp8,
                           kind="ExternalInput")
    rhs8 = nc.dram_tensor("rhs8", [128, 2 * SHARD], fp8,
                          kind="ExternalInput")
    biasr = nc.dram_tensor("biasr", [128, RJT], f32,
                           kind="ExternalInput")

    o_genv = nc.dram_tensor("o_genv", [128, NJT * NBLK], f32,
                            kind="ExternalOutput")
    o_reals = nc.dram_tensor("o_reals", [128, RJT * NIT], f32,
                             kind="ExternalOutput")
    o_realv = nc.dram_tensor("o_realv", [128, NVR * NBLK], f32,
                             kind="ExternalOutput")

    with TileContext(nc) as tc:
        with (
            tc.tile_pool(name="const", bufs=1) as constp,
            tc.tile_pool(name="lhs", bufs=6) as lhsp,
            tc.tile_pool(name="junk", bufs=3) as junkp,
            tc.tile_pool(name="outb", bufs=1) as outp,
            tc.tile_pool(name="psg", bufs=2, space="PSUM") as psgp,
            tc.tile_pool(name="psr", bufs=2, space="PSUM") as psrp,
        ):
            rhs8_sb = constp.tile([128, 2 * SHARD], fp8)
            nc.sync.dma_start(out=rhs8_sb[:, 0:NT], in_=rhs8[:, 0:NT])
            nc.sync.dma_start(out=rhs8_sb[:, SHARD:SHARD + NT],
                              in_=rhs8[:, SHARD:SHARD + NT])
            biasr_sb = constp.tile([128, RJT], f32)
            nc.sync.dma_start(out=biasr_sb[:, :], in_=biasr[:, :])

            genv = outp.tile([128, NJT * NBLK], f32)
            racc = outp.tile([128, RJT * NIT], f32)
            realv = outp.tile([128, NVR * NBLK], f32)
            vslot_idx = {jt: k for k, jt in enumerate(VSLOT)}

            def rhs_ap(io):
                return rhs8_sb[:, :].rearrange(
                    "p (t i) -> p t i", t=2)[:, :, io:io + NT]

            for jt in range(NJT):
                lhs_g = lhsp.tile([128, 2 * JTILE], fp8, tag="lhs_g")
                nc.sync.dma_start(
                    out=lhs_g[:, :],
                    in_=colg8[:, jt * 2 * JTILE:(jt + 1) * 2 * JTILE],
                )
                lhs_r = lhsp.tile([128, 2 * JTILE], fp8, tag="lhs_r")
                nc.sync.dma_start(
                    out=lhs_r[:, :],
                    in_=colr8[:, jt * 2 * JTILE:(jt + 1) * 2 * JTILE],
                )
                if jt == 0:
                    # remaining const slices, behind jt0's critical loads
                    for it0 in range(1, NIT):
                        io0 = it0 * NT
                        nc.sync.dma_start(out=rhs8_sb[:, io0:io0 + NT],
                                          in_=rhs8[:, io0:io0 + NT])
                        nc.sync.dma_start(
                            out=rhs8_sb[:, SHARD + io0:SHARD + io0 + NT],
                            in_=rhs8[:, SHARD + io0:SHARD + io0 + NT])

                # gen: one 3-bank PSUM tile, reduced PSUM-direct by the DVE
                ps_g = psgp.tile([128, SHARD], f32, tag="ps_g")
                for it in range(NIT):
                    io = it * NT
                    nc.tensor.matmul(
                        out=ps_g[:, io:io + NT],
                        lhsT=lhs_g[:, :].rearrange("p (t j) -> p t j", t=2),
                        rhs=rhs_ap(io),
                        start=True, stop=True,
                        perf_mode=mybir.MatmulPerfMode.DoubleRow,
                    )
                    ps_r = psrp.tile([128, NT], f32, tag="ps_r")
                    nc.tensor.matmul(
                        out=ps_r[:, :],
                        lhsT=lhs_r[:, :].rearrange("p (t j) -> p t j", t=2),
                        rhs=rhs_ap(io),
                        start=True, stop=True,
                        perf_mode=mybir.MatmulPerfMode.DoubleRow,
                    )
                    if jt in vslot_idx:
                        # DVE path: per-128-block maxima of X (PSUM-direct);
                        # the per-q |r_q|^2 shift is applied on the host
                        vo = (vslot_idx[jt] * NIT + it) * 4
                        nc.vector.tensor_reduce(
                            out=realv[:, vo:vo + 4],
                            in_=ps_r[:, :].rearrange("p (b x) -> p b x", b=4),
                            axis=mybir.AxisListType.X,
                            op=mybir.AluOpType.max)
                    else:
                        # acc[q] = sum_i exp((X - |r_q|^2 + C0)/T), PSUM-direct
                        junk = junkp.tile([128, NT], f32, tag="junk")
                        nc.scalar.activation(
                            out=junk[:, :],
                            in_=ps_r[:, :],
                            func=mybir.ActivationFunctionType.Exp,
                            bias=biasr_sb[:, jt:jt + 1],
                            scale=1.0 / TSM,
                            accum_out=racc[:, jt * NIT + it:jt * NIT + it + 1],
                        )

                # per-128-block maxima of X, straight out of PSUM
                nc.vector.tensor_reduce(
                    out=genv[:, jt * NBLK:(jt + 1) * NBLK],
                    in_=ps_g[:, :].rearrange("p (b x) -> p b x", b=NBLK),
                    axis=mybir.AxisListType.X,
                    op=mybir.AluOpType.max)

            nc.sync.dma_start(out=o_genv[:, :], in_=genv[:, :])
            nc.sync.dma_start(out=o_reals[:, :], in_=racc[:, :])
            nc.sync.dma_start(out=o_realv[:, :], in_=realv[:, :])

    nc.compile()
    return nc


def _dr_pack(featT, f8, norm_hi, norm_lo):
    """[256-K, C] f32 -> fp8 DoubleRow [128, (tile, t, col)] layout.

    Rows 254/255 get the scaled norm hi/lo (rhs side) or the NSCALE
    constant (lhs side).
    """
    Dd, C = featT.shape
    assert Dd == D and C % JTILE == 0
    nt_ = C // JTILE
    a = featT.copy()
    a[DE] = norm_hi if norm_hi is not None else NSCALE
    a[DE + 1] = norm_lo if norm_lo is not None else NSCALE
    out = (a.reshape(2, 128, nt_, JTILE).transpose(1, 2, 0, 3)
           .reshape(128, nt_ * 2 * JTILE))
    return np.ascontiguousarray(out).astype(f8)


def kernel(real_stats, gen_stats, _trace=False):
    import ml_dtypes
    from concourse.bass_utils import run_bass_kernel_spmd

    f8 = ml_dtypes.float8_e4m3
    global _cached_nc
    real = np.ascontiguousarray(np.asarray(real_stats, dtype=np.float32))
    gen = np.ascontiguousarray(np.asarray(gen_stats, dtype=np.float32))

    realT = np.ascontiguousarray(real.T)                  # [D, N]
    genT = np.ascontiguousarray(gen.T)
    b2 = np.sum(real.astype(np.float64) ** 2, axis=1).astype(np.float32)
    a2g = np.sum(gen.astype(np.float64) ** 2, axis=1).astype(np.float32)

    colg8_np = _dr_pack(genT, f8, None, None)

    in_maps = []
    for c in range(NCORES):
        sl = slice(c * SHARD, (c + 1) * SHARD)
        t = -b2[sl] / NSCALE
        hi = t.astype(f8)
        lo = (t - hi.astype(np.float32)).astype(f8)
        rhs_full = 2.0 * realT[:, sl]
        rhs_full[DE] = hi.astype(np.float32)
        rhs_full[DE + 1] = lo.astype(np.float32)
        rhs8_np = np.ascontiguousarray(
            rhs_full.reshape(2, 128, SHARD).transpose(1, 0, 2)
            .reshape(128, 2 * SHARD)).astype(f8)
        colr_rot = np.roll(realT, -c * SHARD, axis=1)     # full rotation
        colr8_np = _dr_pack(colr_rot, f8, None, None)
        b2rot = np.roll(b2, -c * SHARD)
        biasr_np = np.ascontiguousarray(
            ((C0 - b2rot) / TSM).reshape(RJT, 128).T)     # [128, RJT]
        in_maps.append({
            "colg8": colg8_np,
            "colr8": colr8_np,
            "rhs8": rhs8_np,
            "biasr": biasr_np.astype(np.float32),
        })

    if _cached_nc is None:
        _cached_nc = _build_nc()
    res = run_bass_kernel_spmd(_cached_nc, in_maps,
                               core_ids=list(range(NCORES)),
                               trace=_trace)

    # ---- host combine ----
    NB = NCORES * NBLK                                    # 96 128-blocks
    sslot = np.array([jt for jt in range(NJT) if jt not in set(VSLOT)])
    vslot = np.array(VSLOT)
    # real: smooth-min partials -> coarse d^2 per (real, 512-block)
    d2s = np.full((N, NSB), np.inf, dtype=np.float32)
    d2v128 = np.full((N, NB), np.inf, dtype=np.float32)
    for c in range(NCORES):
        acc = res.results[c]["o_reals"].reshape(128, RJT, NIT)[:, sslot, :]
        with np.errstate(divide="ignore", invalid="ignore"):
            part = C0 - TSM * np.log(acc)                 # [128, nS, NIT]
        part = np.where(np.isfinite(part), part, np.inf).astype(np.float32)
        q = (c * SHARD + sslot[None, :, None] * JTILE
             + np.arange(128)[:, None, None]) % N
        sb = c * NIT + np.arange(NIT)[None, None, :]
        idx = (q * NSB + sb).ravel()
        np.minimum.at(d2s.ravel(), idx, part.ravel())
        # DVE-scanned real tiles: exact-ish 128-block partials of X
        rv = res.results[c]["o_realv"].reshape(128, NVR, NBLK)
        qv = (c * SHARD + vslot[None, :, None] * JTILE
              + np.arange(128)[:, None, None]) % N        # [128, NVR, 1]
        d2p = b2[qv] - rv                                 # d^2 partial
        gb = c * NBLK + np.arange(NBLK)[None, None, :]
        idxv = (qv * NB + gb).ravel()
        np.minimum.at(d2v128.ravel(), idxv, d2p.ravel())
    diag_sb = np.arange(N) // NT
    d2s_m = d2s.copy()
    d2s_m[np.arange(N), diag_sb] = np.inf                 # mask diag block
    d2v128[np.arange(N), np.arange(N) // JTILE] = np.inf  # mask diag block

    # gen: coarse block maxima of X = 2g.r - |r|^2
    Xb = np.empty((NB, N), dtype=np.float32)
    for c in range(NCORES):
        gv = res.results[c]["o_genv"].reshape(128, NJT, NBLK)
        Xb[c * NBLK:(c + 1) * NBLK, :] = (
            gv.transpose(2, 1, 0).reshape(NBLK, N))
    best = Xb.max(axis=0)
    cand_mask = Xb >= (best - MARGIN_G)[None, :]          # [96, N]
    Xstar = np.full(N, -np.inf, dtype=np.float32)
    istar = np.zeros(N, dtype=np.int64)
    for g in range(NB):
        js = np.nonzero(cand_mask[g])[0]
        if js.size == 0:
            continue
        rb = real[g * JTILE:(g + 1) * JTILE]              # [128, D]
        Xex = 2.0 * (gen[js] @ rb.T) - b2[g * JTILE:(g + 1) * JTILE][None, :]
        loc = np.argmax(Xex, axis=1)
        val = Xex[np.arange(js.size), loc]
        upd = val > Xstar[js]
        Xstar[js[upd]] = val[upd]
        istar[js[upd]] = g * JTILE + loc[upd]
    d1 = np.sqrt(np.maximum(a2g - Xstar, 0.0))

    # realNN: exact refinement only at the used indices
    used = np.unique(istar)
    du = d2s_m[used]                                      # [U, 24]
    duv = d2v128[used]                                    # [U, 96]
    coarse = np.minimum(du.min(axis=1), duv.min(axis=1))
    rcand = du <= (coarse + MARGIN_R)[:, None]
    rcand[~np.isfinite(coarse)] = True                    # fallback: all
    rcand[np.arange(used.size), diag_sb[used]] = True     # always diag
    rcandv = duv <= (coarse + MARGIN_RV)[:, None]         # [U, 96]
    nn2 = np.full(used.size, np.inf, dtype=np.float32)
    for g in range(NSB):
        rs = np.nonzero(rcand[:, g])[0]
        if rs.size == 0:
            continue
        ridx = used[rs]
        rb = real[g * NT:(g + 1) * NT]
        d2 = (b2[ridx][:, None] + b2[g * NT:(g + 1) * NT][None, :]
              - 2.0 * (real[ridx] @ rb.T))
        inblk = (ridx >= g * NT) & (ridx < (g + 1) * NT)
        d2[inblk, ridx[inblk] - g * NT] = np.inf          # exclude self
        nn2[rs] = np.minimum(nn2[rs], d2.min(axis=1))
    for g in range(NB):
        rs = np.nonzero(rcandv[:, g] & ~rcand[:, g // 4])[0]
        if rs.size == 0:
            continue
        ridx = used[rs]
        rb = real[g * JTILE:(g + 1) * JTILE]
        d2 = (b2[ridx][:, None] + b2[g * JTILE:(g + 1) * JTILE][None, :]
              - 2.0 * (real[ridx] @ rb.T))
        inblk = (ridx >= g * JTILE) & (ridx < (g + 1) * JTILE)
        d2[inblk, ridx[inblk] - g * JTILE] = np.inf       # exclude self
        nn2[rs] = np.minimum(nn2[rs], d2.min(axis=1))
    lut = np.zeros(N, dtype=np.float32)
    lut[used] = np.sqrt(np.maximum(nn2, 0.0))
    d2v = lut[istar]

    z = (d2v - d1) / 0.1
    authen = np.where(z >= 0, 1.0 / (1.0 + np.exp(-np.abs(z))),
                      np.exp(-np.abs(z)) / (1.0 + np.exp(-np.abs(z))))
    out = np.asarray(-100.0 * np.mean(authen), dtype=np.float32)
    if _trace:
        return out, res
    return out
